# revision 17
# baseline (speedup 1.0000x reference)
"""Trainium2 Bass kernel for MultiHeadDifferentialAttention.

Strategy: data-parallel over batch. B=8 batches map 1:1 onto the 8
NeuronCores; each core runs the full per-batch pipeline (QKV proj ->
differential attention -> LayerNorm -> output proj) with no collectives.
The host pre-lays-out inputs (x transposed per batch, weights reshaped
into partition-major tiles, gamma/beta/0.8 folded into Wp/bp) and
transposes the per-core [768, 1024] outputs back at the end.

Device pipeline per core:
  - v = x @ Wv (fp16 operands, fp32 accum) into an augmented layout
    [tok, head, 128+1] whose last column is ones, so the attention-value
    matmul also produces the softmax denominator (column 128) for free.
  - qT/kT = (x @ Wq)^T per head in [2D=128, tok] fp16 layout: q1/q2 land
    on partitions 0-63 / 64-127, so the two K=64 score matmuls pack into
    disjoint PE row groups and run concurrently (they must target
    different PSUM banks - concurrent same-bank PE writes fault).
  - scores S^T[m, n] on PSUM -> one strided exp per m on ScalarE (scale
    fused) -> fp16 E tiles.
  - AV: E tile is the stationary operand, rhs = [v_h | 1]; out[n, 0:128]
    is the unnormalized attention output, out[:, 128] the denominator.
    The two scores' accumulation chains share one PSUM bank
    (only the first matmul carries start=True - start clears the
    has_written bits bank-wide) and run un-interleaved so LDW/MM pairs
    pipeline.
  - combine a1 - lam*a2 and LayerNorm on VectorE, all per-partition.
    rsqrt = exp(-0.5*ln(var+eps)) on ScalarE: the activation-table patch
    below pins exp and ln to the one table set containing both, so the
    per-head LayerNorm causes no table reloads. The finished head is
    immediately PE-transposed into the [1536, tok] layout the final
    f32r projection consumes. Output is F^T [768, 1024].
"""

import numpy as np

B, N, C, H = 8, 1024, 768, 12
D = C // H  # 64
TD = 2 * D  # 128
LAMBDA_INIT = 0.8 - 0.6 * np.exp(-0.3 * (1 - 1))  # 0.2
OUT_SCALE = 1.0 - LAMBDA_INIT  # 0.8
EPS = 1e-5
SCALE = float(D) ** -0.5  # 1/8

_BUILD_CACHE = {}
LAST_EXEC_NS = None
LAST_RESULTS = None


def _patch_act_tables(mybir, bacc):
    """Pin Exp and Ln to natural_log_exp_and_others so interleaving them
    never reloads the ScalarE spline tables."""
    from concourse import hw_specs

    orig = hw_specs.get_activation_tables
    if getattr(bacc.get_activation_tables, "_nlx_pinned", False):
        return

    def patched(arch):
        tables = orig(arch)
        exp = mybir.ActivationFunctionType.Exp
        ln = mybir.ActivationFunctionType.Ln
        for name, funcs in tables.items():
            if name != "natural_log_exp_and_others":
                funcs.discard(exp)
                funcs.discard(ln)
        return tables

    patched._nlx_pinned = True
    bacc.get_activation_tables = patched


def _build(lam: float, dbg: bool = False):
    import concourse.bass as bass  # noqa: F401
    import concourse.mybir as mybir
    import concourse.tile as tile
    from concourse import bacc
    from concourse.masks import make_identity

    _patch_act_tables(mybir, bacc)

    f32 = mybir.dt.float32
    f32r = mybir.dt.float32r
    f16 = mybir.dt.float16
    AF = mybir.ActivationFunctionType
    OP = mybir.AluOpType

    nc = bacc.Bacc(None, target_bir_lowering=False, debug=False)

    XT = nc.declare_dram_parameter("xT", [128, 6, 1024], f16, isOutput=False)
    WQR = nc.declare_dram_parameter("WqR", [12, 128, 6, 128], f16, isOutput=False)
    WKR = nc.declare_dram_parameter("WkR", [12, 128, 6, 128], f16, isOutput=False)
    WVR = nc.declare_dram_parameter("WvR", [128, 6, 1536], f16, isOutput=False)
    WPR = nc.declare_dram_parameter("WpR", [12, 128, 768], f16, isOutput=False)
    BPP = nc.declare_dram_parameter("bpp", [128, 6], f32, isOutput=False)
    OUT = nc.declare_dram_parameter("outT", [128, 6, 1024], f32, isOutput=True)
    if dbg:
        DVAUG = nc.declare_dram_parameter("d_vaug", [128, 8, 12, 129], f16, isOutput=True)
        DQH = nc.declare_dram_parameter("d_qh", [128, 1024], f16, isOutput=True)
        DKH = nc.declare_dram_parameter("d_kh", [128, 1024], f16, isOutput=True)
        DE12 = nc.declare_dram_parameter("d_e12", [128, 8, 1024], f16, isOutput=True)
        DOLN = nc.declare_dram_parameter("d_oln", [128, 8, 12, 128], f16, isOutput=True)
        DSTATS = nc.declare_dram_parameter("d_stats", [128, 12, 8, 2], f32, isOutput=True)
        DOLNT = nc.declare_dram_parameter("d_olnT", [128, 12, 1024], f16, isOutput=True)

    with tile.TileContext(nc) as tc:
        with tc.tile_pool(name="persist", bufs=1) as persist:
            o_ln = persist.tile([128, 8, 12, 128], f16)
            o_lnT = persist.tile([128, 12, 1024], f16)
            stats_sb = persist.tile([128, 12, 8, 2], f32)
            sdbuf = persist.tile([128, 12, 8], f32)
            rsbuf = persist.tile([128, 12, 8], f32)
            ident = persist.tile([128, 128], f16)
            bpp_sb = persist.tile([128, 6], f32)
            eps_sb = persist.tile([128, 1], f32)
            with (
                tc.tile_pool(name="wps", bufs=1) as wpsp,
                tc.tile_pool(name="longA", bufs=1) as longA,
            ):
                xTk = [
                    longA.tile([128, 1024], f16, name=f"xT{k}") for k in range(6)
                ]
                v_aug = longA.tile([128, 8, 12, 129], f16)
                nc.vector.memset(v_aug[:, :, :, 128:129], 1.0)
                make_identity(nc, ident[:])
                nc.sync.dma_start(out=bpp_sb[:], in_=BPP[:])
                nc.vector.memset(eps_sb[:], EPS)

                from contextlib import ExitStack as _ES
                _pools = _ES()
                wqkp = _pools.enter_context(tc.tile_pool(name="wqk", bufs=3))
                qkp = _pools.enter_context(tc.tile_pool(name="qk", bufs=3))
                qkps = _pools.enter_context(
                    tc.tile_pool(name="qkps", bufs=1, space="PSUM")
                )

                def qk_ops(h):
                    """q^T/k^T projection for head h as a list of small
                    closures (one 512-col matmul or one psum->sbuf copy
                    each) so they can be spread across the paced slots of
                    the attention interleave.  The four 512-col chains share
                    ONE psum bank sequentially; each chain's start=True
                    clear is gated on the previous chain's copy, and the
                    gaps hide behind the surrounding AV/score matmuls."""
                    wqh = wqkp.tile([128, 6, 128], f16, tag="wq",
                                    name=f"wqh{h}")
                    wkh = wqkp.tile([128, 6, 128], f16, tag="wk",
                                    name=f"wkh{h}")
                    nc.sync.dma_start(out=wqh[:], in_=WQR[h])
                    nc.sync.dma_start(out=wkh[:], in_=WKR[h])
                    qh = qkp.tile([128, 1024], f16, tag="q", name=f"qh{h}")
                    kh = qkp.tile([128, 1024], f16, tag="k", name=f"kh{h}")
                    ops = []
                    for wt, dst in ((wqh, qh), (wkh, kh)):
                        for half in range(2):
                            ps = qkps.tile([128, 512], f32, tag="qk",
                                           name="qkps")
                            nsl = slice(half * 512, (half + 1) * 512)
                            for k in range(6):
                                def mm(wt=wt, ps=ps, k=k, nsl=nsl):
                                    nc.tensor.matmul(
                                        ps[:], wt[:, k, :], xTk[k][:, nsl],
                                        start=(k == 0), stop=(k == 5),
                                    )
                                ops.append(mm)
                            def cp(dst=dst, ps=ps, nsl=nsl):
                                nc.vector.tensor_copy(dst[:, nsl], ps[:])
                            ops.append(cp)
                    return qh, kh, ops

                # ---- Phase 1: v = x @ Wv into v_aug ----
                with (
                    tc.tile_pool(name="wv", bufs=1) as wvp,
                    tc.tile_pool(name="vps", bufs=2, space="PSUM") as vps,
                ):
                    wvk = [
                        wvp.tile([128, 1536], f16, name=f"wv{k}")
                        for k in range(6)
                    ]
                    # x chunks first so the q/k matmuls (which need all six)
                    # can start as early as possible; wv lands while the q/k
                    # projection runs.
                    for k in range(6):
                        nc.sync.dma_start(out=xTk[k][:], in_=XT[:, k])
                    qh0, kh0, ops0 = qk_ops(0)
                    next_qk = (qh0, kh0)
                    for k in range(6):
                        nc.sync.dma_start(out=wvk[k][:], in_=WVR[:, k])
                    for t in range(8):
                        # one stationary xT chunk serves all three c-ranges
                        pss = [
                            vps.tile([128, 512], f32, tag=f"vps{cr}",
                                     name=f"vps{cr}")
                            for cr in range(3)
                        ]
                        for k in range(6):
                            for cr in range(3):
                                nc.tensor.matmul(
                                    pss[cr][:],
                                    xTk[k][:, t * 128 : (t + 1) * 128],
                                    wvk[k][:, cr * 512 : (cr + 1) * 512],
                                    start=(k == 0),
                                    stop=(k == 5),
                                )
                        # head-0 q/k chains woven between the v chunks so
                        # their single-psum-bank serialization hides
                        for _ in range(4):
                            if ops0:
                                ops0.pop(0)()
                        for cr in range(3):
                            nc.scalar.copy(
                                v_aug[:, t, 4 * cr : 4 * cr + 4, 0:128],
                                pss[cr][:].rearrange("p (h c) -> p h c", c=128),
                            )
                    while ops0:
                        ops0.pop(0)()

                # Prefetch all 12 final-projection weight tiles now; the DMA
                # engines are idle through the attention phase and the tiles
                # stay resident so the tail never waits on weight DMA.
                wpks = []
                for k in range(12):
                    wpk = wpsp.tile([128, 768], f16, tag=f"wp{k}",
                                    name=f"wpk{k}")
                    nc.sync.dma_start(out=wpk[:], in_=WPR[k])
                    wpks.append(wpk)

                # ---- Phase 2: attention, strip-level software pipeline ----
                # ScalarE's exp throughput (~16.7us/head) nearly equals the
                # PE's per-head matmul work (~17.6us), so neither engine can
                # afford to wait on the other.  AV chains run m-outer,
                # consuming each exp'd strip as it lands, interleaved with
                # the next strip-set's score pairs and the next head's q/k
                # projection chains; AV tiles c2 in {2,3} run as catch-up
                # blocks with no exp dependency left.
                with (
                    tc.tile_pool(name="estrip", bufs=3) as ep,
                    tc.tile_pool(name="fin", bufs=4) as fin,
                    tc.tile_pool(name="spool", bufs=2, space="PSUM") as spool,
                    tc.tile_pool(name="avps", bufs=3, space="PSUM") as avps,
                ):

                    def s_strip(qh, kh, e12, r, m):
                        """Score pair + exp for strip (r, m).  The two score
                        matmuls must hit different PSUM banks (concurrent
                        row-group writes to one bank fault); one exp covers
                        both."""
                        nsl = slice(r * 512, (r + 1) * 512)
                        msl = slice(m * 128, (m + 1) * 128)
                        sp = spool.tile([128, 2, 512], f32, tag="s")
                        nc.tensor.matmul(
                            sp[:, 0, :], kh[0:64, msl], qh[0:64, nsl],
                            start=True, stop=True,
                        )
                        nc.tensor.matmul(
                            sp[:, 1, :], kh[64:128, msl], qh[64:128, nsl],
                            start=True, stop=True,
                        )
                        nc.scalar.activation(
                            e12[:, m, :].rearrange("p (a b) -> p a b", a=2),
                            sp[:],
                            AF.Exp,
                            scale=SCALE,
                        )

                    def av_strip(h, o, e12, m, c2):
                        """One m-strip of both accumulation chains for AV
                        tile c2.  Both chains share one PSUM bank: only the
                        E1 chain's first matmul carries start=True (clears
                        has_written bank-wide); the E2 chain's first write
                        still overwrites because its bits are already
                        clear."""
                        nc.tensor.matmul(
                            o[:, 0:129],
                            e12[:, m, c2 * 128 : (c2 + 1) * 128],
                            v_aug[:, m, h, :],
                            start=(m == 0),
                            stop=(m == 7),
                            skip_group_check=True,
                        )
                        nc.tensor.matmul(
                            o[:, 129:258],
                            e12[:, m, 512 + c2 * 128 : 512 + (c2 + 1) * 128],
                            v_aug[:, m, h, :],
                            start=False,
                            stop=(m == 7),
                            skip_group_check=True,
                        )

                    def combine(h, r, c2, o):
                        """a1 - lam*a2 + LN stats for one finished AV tile
                        (VectorE, all per-partition)."""
                        jn = r * 4 + c2
                        r1 = fin.tile([128, 1], f32, tag="r1")
                        r2 = fin.tile([128, 1], f32, tag="r2")
                        nc.vector.reciprocal(r1[:], o[:, 128:129])
                        nc.vector.reciprocal(r2[:], o[:, 257:258])
                        t2 = fin.tile([128, 128], f32, tag="t2")
                        nc.vector.tensor_scalar(
                            t2[:], o[:, 129:257], r2[:], float(lam),
                            op0=OP.mult, op1=OP.mult,
                        )
                        nc.vector.scalar_tensor_tensor(
                            o_ln[:, jn, h, :],
                            o[:, 0:128],
                            r1[:],
                            t2[:],
                            op0=OP.mult,
                            op1=OP.subtract,
                        )
                        st6 = fin.tile([128, 6], f32, tag="st6")
                        nc.vector.bn_stats(st6[:], o_ln[:, jn, h, :])
                        nc.vector.bn_aggr(stats_sb[:, h, jn, :], st6[:])

                    # prologue: head 0's first strip-set
                    qh, kh = next_qk
                    e_r0 = ep.tile([128, 8, 1024], f16, tag="e")
                    for m in range(8):
                        s_strip(qh, kh, e_r0, 0, m)
                    if dbg:
                        nc.sync.dma_start(out=DE12[:], in_=e_r0[:])

                    for h in range(12):
                        if h + 1 < 12:
                            qh_n, kh_n, qk_chunks = qk_ops(h + 1)
                            while qk_chunks:  # BISECT: no spreading
                                qk_chunks.pop(0)()
                        else:
                            qh_n = kh_n = None
                            qk_chunks = []

                        # loop A: r1 scores + AV(r0) tiles {0,1} + qk chains
                        e_r1 = ep.tile([128, 8, 1024], f16, tag="e")
                        o_a = [
                            avps.tile([128, 258], f32, tag="o", name="o_av")
                            for _ in range(2)
                        ]
                        for m in range(8):
                            for c2 in range(2):
                                av_strip(h, o_a[c2], e_r0, m, c2)
                            for _ in range(2):
                                if qk_chunks:
                                    qk_chunks.pop(0)()
                            s_strip(qh, kh, e_r1, 1, m)
                        combine(h, 0, 0, o_a[0])
                        combine(h, 0, 1, o_a[1])
                        # catch-up: AV(r0) tiles {2,3} (exp all done)
                        o_b = [
                            avps.tile([128, 258], f32, tag="o", name="o_av")
                            for _ in range(2)
                        ]
                        for m in range(8):
                            for c2 in range(2):
                                av_strip(h, o_b[c2], e_r0, m, c2 + 2)
                        combine(h, 0, 2, o_b[0])
                        combine(h, 0, 3, o_b[1])

                        # loop B: next head's r0 scores + AV(r1) tiles {0,1}
                        e_n = None
                        if qh_n is not None:
                            e_n = ep.tile([128, 8, 1024], f16, tag="e")
                        o_c = [
                            avps.tile([128, 258], f32, tag="o", name="o_av")
                            for _ in range(2)
                        ]
                        for m in range(8):
                            for _ in range(2):
                                if qk_chunks:
                                    qk_chunks.pop(0)()
                            for c2 in range(2):
                                av_strip(h, o_c[c2], e_r1, m, c2)
                            if e_n is not None:
                                s_strip(qh_n, kh_n, e_n, 0, m)
                        combine(h, 1, 0, o_c[0])
                        combine(h, 1, 1, o_c[1])
                        # catch-up: AV(r1) tiles {2,3}
                        o_d = [
                            avps.tile([128, 258], f32, tag="o", name="o_av")
                            for _ in range(2)
                        ]
                        for m in range(8):
                            for c2 in range(2):
                                av_strip(h, o_d[c2], e_r1, m, c2 + 2)
                        combine(h, 1, 2, o_d[0])
                        combine(h, 1, 3, o_d[1])

                        if dbg and h == 0:
                            nc.sync.dma_start(out=DQH[:], in_=qh[:])
                            nc.sync.dma_start(out=DKH[:], in_=kh[:])

                        # ---- per-head tail: rsqrt + LN apply ----
                        # rs = exp(-0.5 * ln(var + eps)); Exp and Ln share one
                        # pinned table set, so no reload happens here.
                        nc.scalar.activation(
                            sdbuf[:, h, :], stats_sb[:, h, :, 1],
                            AF.Ln, bias=eps_sb[:],
                        )
                        nc.scalar.activation(
                            rsbuf[:, h, :], sdbuf[:, h, :], AF.Exp, scale=-0.5
                        )
                        for jn in range(8):
                            nc.vector.tensor_scalar(
                                o_ln[:, jn, h, :],
                                o_ln[:, jn, h, :],
                                stats_sb[:, h, jn, 0:1],
                                rsbuf[:, h, jn : jn + 1],
                                op0=OP.subtract,
                                op1=OP.mult,
                            )

                        e_r0 = e_n
                        qh, kh = qh_n, kh_n
                    if dbg:
                        nc.sync.dma_start(out=DVAUG[:], in_=v_aug[:])
                        nc.sync.dma_start(out=DOLN[:], in_=o_ln[:])
                        nc.sync.dma_start(out=DSTATS[:], in_=stats_sb[:])

                _pools.close()

                # ---- tail: transposes interleaved with the projection.
                # Head k+1's transposes (LDW-bound) are emitted right after
                # head k's projection matmuls so their weight loads hide in
                # the PE reorder window; o_lnT[k+1] is ready (via the ScalarE
                # psum->sbuf copy) before the k+1 projection step needs it.
                if dbg:
                    nc.sync.dma_start(out=DOLNT[:], in_=o_lnT[:])
                with (
                    tc.tile_pool(name="tps", bufs=2, space="PSUM") as tps,
                    tc.tile_pool(name="tail", bufs=1) as tailp,
                    tc.tile_pool(name="fps", bufs=1, space="PSUM") as fps,
                ):
                    fout = tailp.tile([128, 6, 1024], f32)

                    def transpose_head(h):
                        for g2 in range(2):
                            tp = tps.tile([128, 4, 128], f16, tag="t")
                            for j in range(4):
                                jn = 4 * g2 + j
                                nc.tensor.transpose(
                                    tp[:, j, :], o_ln[:, jn, h, :], ident[:]
                                )
                            nc.scalar.copy(
                                o_lnT[:, h, g2 * 512 : (g2 + 1) * 512],
                                tp[:].rearrange("p a b -> p (a b)"),
                            )

                    transpose_head(0)
                    for g in range(2):
                        fs = {}
                        for mc in range(3 * g, 3 * g + 3):
                            for nr2 in range(2):
                                fs[(mc, nr2)] = fps.tile(
                                    [128, 512], f32, tag=f"f{mc % 3}_{nr2}",
                                    name=f"fpsum{mc}_{nr2}",
                                )
                        for k in range(12):
                            for mc in range(3 * g, 3 * g + 3):
                                for nr2 in range(2):
                                    nc.tensor.matmul(
                                        fs[(mc, nr2)][:],
                                        wpks[k][:, mc * 128 : (mc + 1) * 128],
                                        o_lnT[:, k, nr2 * 512 : (nr2 + 1) * 512],
                                        start=(k == 0),
                                        stop=(k == 11),
                                    )
                            if g == 0 and k + 1 < 12:
                                transpose_head(k + 1)
                        for mc in range(3 * g, 3 * g + 3):
                            for nr2 in range(2):
                                nsl2 = slice(nr2 * 512, (nr2 + 1) * 512)
                                nc.vector.tensor_scalar(
                                    fout[:, mc, nsl2],
                                    fs[(mc, nr2)][:],
                                    bpp_sb[:, mc : mc + 1],
                                    None,
                                    op0=OP.add,
                                )
                                nc.sync.dma_start(
                                    out=OUT[:, mc, nsl2],
                                    in_=fout[:, mc, nsl2],
                                )

    nc.compile()
    return nc


def _host_prep(x, Wq, Wk, Wv, gamma, beta, Wp, bp):
    x = np.ascontiguousarray(np.asarray(x, np.float32))
    Wq = np.asarray(Wq, np.float32)
    Wk = np.asarray(Wk, np.float32)
    Wv = np.asarray(Wv, np.float32)
    Wp = np.asarray(Wp, np.float32)
    bp = np.asarray(bp, np.float32)
    gamma = np.asarray(gamma, np.float32)
    beta = np.asarray(beta, np.float32)

    # xT per batch: [128, 6, 1024] with [p, k, n] = x[b, n, k*128+p]
    xTr = np.ascontiguousarray(
        x.transpose(0, 2, 1).reshape(B, 6, 128, N).transpose(0, 2, 1, 3)
    ).astype(np.float16)

    # W[qk]R: [12, 128, 6, 128] with [h, p, k, c] = W[k*128+p, h*128+c]
    def wqk_r(W):
        return np.ascontiguousarray(
            W.reshape(6, 128, 12, 128).transpose(2, 1, 0, 3)
        )

    WqR = wqk_r(Wq).astype(np.float16)
    WkR = wqk_r(Wk).astype(np.float16)
    # WvR: [128, 6, 1536] with [p, k, c] = Wv[k*128+p, c]
    WvR = np.ascontiguousarray(
        Wv.reshape(6, 128, 2 * C).transpose(1, 0, 2)
    ).astype(np.float16)
    # Fold gamma and the (1 - lambda_init) scale into Wp; beta into the bias.
    gfull = np.tile(gamma, H)  # [1536]
    Wpg = Wp * (OUT_SCALE * gfull)[:, None]
    bpp = bp + OUT_SCALE * (np.tile(beta, H) @ Wp)
    WpR = np.ascontiguousarray(Wpg.reshape(12, 128, C)).astype(np.float16)
    bppR = np.ascontiguousarray(bpp.reshape(6, 128).T)  # [128, 6]
    return xTr, WqR, WkR, WvR, WpR, bppR


def kernel(x, Wq, Wk, Wv, lam, gamma, beta, Wp, bp):
    global LAST_EXEC_NS, LAST_RESULTS
    import os

    from concourse.bass_utils import run_bass_kernel_spmd

    lam_f = float(np.asarray(lam))
    xTr, WqR, WkR, WvR, WpR, bppR = _host_prep(
        x, Wq, Wk, Wv, gamma, beta, Wp, bp
    )

    key = lam_f
    if key not in _BUILD_CACHE:
        _BUILD_CACHE[key] = _build(lam_f)
    nc = _BUILD_CACHE[key]

    in_maps = [
        {
            "xT": xTr[b],
            "WqR": WqR,
            "WkR": WkR,
            "WvR": WvR,
            "WpR": WpR,
            "bpp": bppR,
        }
        for b in range(B)
    ]

    trace = bool(os.environ.get("BASS_KERNEL_TRACE"))
    if trace:
        from concourse import bass_utils as _bu

        _bu.upload_artifacts = lambda tmpdir: "local://" + tmpdir
    res = run_bass_kernel_spmd(
        nc, in_maps, list(range(B)), trace=trace,
        **({"trace_cores": list(range(B))} if trace else {}),
    )
    LAST_EXEC_NS = res.exec_time_ns
    LAST_RESULTS = res

    out = np.empty((B, N, C), np.float32)
    for b in range(B):
        outT = res.results[b]["outT"]  # [128, 6, 1024]
        out[b] = outT.transpose(2, 1, 0).reshape(N, C)
    return out



# revision 20
# speedup vs baseline: 1.0385x; 1.0385x over previous
"""Trainium2 Bass kernel for MultiHeadDifferentialAttention.

Strategy: data-parallel over batch. B=8 batches map 1:1 onto the 8
NeuronCores; each core runs the full per-batch pipeline (QKV proj ->
differential attention -> LayerNorm -> output proj) with no collectives.
The host pre-lays-out inputs (x transposed per batch, weights reshaped
into partition-major tiles, gamma/beta/0.8 folded into Wp/bp) and
transposes the per-core [768, 1024] outputs back at the end.

Device pipeline per core:
  - v = x @ Wv (fp16 operands, fp32 accum) into an augmented layout
    [tok, head, 128+1] whose last column is ones, so the attention-value
    matmul also produces the softmax denominator (column 128) for free.
  - qT/kT = (x @ Wq)^T per head in [2D=128, tok] fp16 layout: q1/q2 land
    on partitions 0-63 / 64-127, so the two K=64 score matmuls pack into
    disjoint PE row groups and run concurrently (they must target
    different PSUM banks - concurrent same-bank PE writes fault).
  - scores S^T[m, n] on PSUM -> one strided exp per m on ScalarE (scale
    fused) -> fp16 E tiles.
  - AV: E tile is the stationary operand, rhs = [v_h | 1]; out[n, 0:128]
    is the unnormalized attention output, out[:, 128] the denominator.
    The two scores' accumulation chains share one PSUM bank
    (only the first matmul carries start=True - start clears the
    has_written bits bank-wide) and run un-interleaved so LDW/MM pairs
    pipeline.
  - combine a1 - lam*a2 and LayerNorm on VectorE, all per-partition.
    rsqrt = exp(-0.5*ln(var+eps)) on ScalarE: the activation-table patch
    below pins exp and ln to the one table set containing both, so the
    per-head LayerNorm causes no table reloads. The finished head is
    immediately PE-transposed into the [1536, tok] layout the final
    f32r projection consumes. Output is F^T [768, 1024].
"""

import numpy as np

B, N, C, H = 8, 1024, 768, 12
D = C // H  # 64
TD = 2 * D  # 128
LAMBDA_INIT = 0.8 - 0.6 * np.exp(-0.3 * (1 - 1))  # 0.2
OUT_SCALE = 1.0 - LAMBDA_INIT  # 0.8
EPS = 1e-5
SCALE = float(D) ** -0.5  # 1/8

_BUILD_CACHE = {}
LAST_EXEC_NS = None
LAST_RESULTS = None


def _patch_act_tables(mybir, bacc):
    """Pin Exp and Ln to natural_log_exp_and_others so interleaving them
    never reloads the ScalarE spline tables."""
    from concourse import hw_specs

    orig = hw_specs.get_activation_tables
    if getattr(bacc.get_activation_tables, "_nlx_pinned", False):
        return

    def patched(arch):
        tables = orig(arch)
        exp = mybir.ActivationFunctionType.Exp
        ln = mybir.ActivationFunctionType.Ln
        for name, funcs in tables.items():
            if name != "natural_log_exp_and_others":
                funcs.discard(exp)
                funcs.discard(ln)
        return tables

    patched._nlx_pinned = True
    bacc.get_activation_tables = patched


def _build(lam: float, dbg: bool = False):
    import concourse.bass as bass  # noqa: F401
    import concourse.mybir as mybir
    import concourse.tile as tile
    from concourse import bacc
    from concourse.masks import make_identity

    _patch_act_tables(mybir, bacc)

    f32 = mybir.dt.float32
    f32r = mybir.dt.float32r
    f16 = mybir.dt.float16
    AF = mybir.ActivationFunctionType
    OP = mybir.AluOpType

    nc = bacc.Bacc(None, target_bir_lowering=False, debug=False)

    XT = nc.declare_dram_parameter("xT", [128, 6, 1024], f16, isOutput=False)
    WQR = nc.declare_dram_parameter("WqR", [12, 128, 6, 128], f16, isOutput=False)
    WKR = nc.declare_dram_parameter("WkR", [12, 128, 6, 128], f16, isOutput=False)
    WVR = nc.declare_dram_parameter("WvR", [128, 6, 1536], f16, isOutput=False)
    WPR = nc.declare_dram_parameter("WpR", [12, 128, 768], f16, isOutput=False)
    BPP = nc.declare_dram_parameter("bpp", [128, 6], f32, isOutput=False)
    OUT = nc.declare_dram_parameter("outT", [128, 6, 1024], f32, isOutput=True)
    if dbg:
        DVAUG = nc.declare_dram_parameter("d_vaug", [128, 8, 12, 129], f16, isOutput=True)
        DQH = nc.declare_dram_parameter("d_qh", [128, 1024], f16, isOutput=True)
        DKH = nc.declare_dram_parameter("d_kh", [128, 1024], f16, isOutput=True)
        DE12 = nc.declare_dram_parameter("d_e12", [128, 8, 1024], f16, isOutput=True)
        DOLN = nc.declare_dram_parameter("d_oln", [128, 8, 12, 128], f16, isOutput=True)
        DSTATS = nc.declare_dram_parameter("d_stats", [128, 12, 8, 2], f32, isOutput=True)
        DOLNT = nc.declare_dram_parameter("d_olnT", [128, 12, 1024], f16, isOutput=True)

    with tile.TileContext(nc) as tc:
        with tc.tile_pool(name="persist", bufs=1) as persist:
            o_ln = persist.tile([128, 8, 12, 128], f16)
            o_lnT = persist.tile([128, 12, 1024], f16)
            stats_sb = persist.tile([128, 12, 8, 2], f32)
            sdbuf = persist.tile([128, 12, 8], f32)
            rsbuf = persist.tile([128, 12, 8], f32)
            ident = persist.tile([128, 128], f16)
            bpp_sb = persist.tile([128, 6], f32)
            eps_sb = persist.tile([128, 1], f32)
            with (
                tc.tile_pool(name="wps", bufs=1) as wpsp,
                tc.tile_pool(name="longA", bufs=1) as longA,
            ):
                xTk = [
                    longA.tile([128, 1024], f16, name=f"xT{k}") for k in range(6)
                ]
                v_aug = longA.tile([128, 8, 12, 129], f16)
                nc.vector.memset(v_aug[:, :, :, 128:129], 1.0)
                make_identity(nc, ident[:])
                nc.sync.dma_start(out=bpp_sb[:], in_=BPP[:])
                nc.vector.memset(eps_sb[:], EPS)

                from contextlib import ExitStack as _ES
                _pools = _ES()
                wqkp = _pools.enter_context(tc.tile_pool(name="wqk", bufs=3))
                qkp = _pools.enter_context(tc.tile_pool(name="qk", bufs=3))
                qkps = _pools.enter_context(
                    tc.tile_pool(name="qkps", bufs=1, space="PSUM")
                )

                def qk_ops(h):
                    """q^T/k^T projection for head h as a list of small
                    closures (one 512-col matmul or one psum->sbuf copy
                    each) so they can be spread across the paced slots of
                    the attention interleave.  The four 512-col chains share
                    ONE psum bank sequentially; each chain's start=True
                    clear is gated on the previous chain's copy, and the
                    gaps hide behind the surrounding AV/score matmuls."""
                    wqh = wqkp.tile([128, 6, 128], f16, tag="wq",
                                    name=f"wqh{h}")
                    wkh = wqkp.tile([128, 6, 128], f16, tag="wk",
                                    name=f"wkh{h}")
                    nc.sync.dma_start(out=wqh[:], in_=WQR[h])
                    nc.sync.dma_start(out=wkh[:], in_=WKR[h])
                    qh = qkp.tile([128, 1024], f16, tag="q", name=f"qh{h}")
                    kh = qkp.tile([128, 1024], f16, tag="k", name=f"kh{h}")
                    ops = []
                    for wt, dst in ((wqh, qh), (wkh, kh)):
                        for half in range(2):
                            ps = qkps.tile([128, 512], f32, tag="qk",
                                           name="qkps")
                            nsl = slice(half * 512, (half + 1) * 512)
                            for k in range(6):
                                def mm(wt=wt, ps=ps, k=k, nsl=nsl):
                                    nc.tensor.matmul(
                                        ps[:], wt[:, k, :], xTk[k][:, nsl],
                                        start=(k == 0), stop=(k == 5),
                                    )
                                ops.append(mm)
                            def cp(dst=dst, ps=ps, nsl=nsl):
                                nc.vector.tensor_copy(dst[:, nsl], ps[:])
                            ops.append(cp)
                    return qh, kh, ops

                # ---- Phase 1: v = x @ Wv into v_aug ----
                with (
                    tc.tile_pool(name="wv", bufs=1) as wvp,
                    tc.tile_pool(name="vps", bufs=2, space="PSUM") as vps,
                ):
                    wvk = [
                        wvp.tile([128, 1536], f16, name=f"wv{k}")
                        for k in range(6)
                    ]
                    # x chunks first so the q/k matmuls (which need all six)
                    # can start as early as possible; wv lands while the q/k
                    # projection runs.
                    for k in range(6):
                        nc.sync.dma_start(out=xTk[k][:], in_=XT[:, k])
                    qh0, kh0, ops0 = qk_ops(0)
                    next_qk = (qh0, kh0)
                    for k in range(6):
                        nc.sync.dma_start(out=wvk[k][:], in_=WVR[:, k])
                    for t in range(8):
                        # one stationary xT chunk serves all three c-ranges
                        pss = [
                            vps.tile([128, 512], f32, tag=f"vps{cr}",
                                     name=f"vps{cr}")
                            for cr in range(3)
                        ]
                        for k in range(6):
                            for cr in range(3):
                                nc.tensor.matmul(
                                    pss[cr][:],
                                    xTk[k][:, t * 128 : (t + 1) * 128],
                                    wvk[k][:, cr * 512 : (cr + 1) * 512],
                                    start=(k == 0),
                                    stop=(k == 5),
                                )
                        # head-0 q/k chains woven between the v chunks so
                        # their single-psum-bank serialization hides
                        for _ in range(4):
                            if ops0:
                                ops0.pop(0)()
                        for cr in range(3):
                            nc.scalar.copy(
                                v_aug[:, t, 4 * cr : 4 * cr + 4, 0:128],
                                pss[cr][:].rearrange("p (h c) -> p h c", c=128),
                            )
                    while ops0:
                        ops0.pop(0)()

                # Prefetch all 12 final-projection weight tiles now; the DMA
                # engines are idle through the attention phase and the tiles
                # stay resident so the tail never waits on weight DMA.
                wpks = []
                for k in range(12):
                    wpk = wpsp.tile([128, 768], f16, tag=f"wp{k}",
                                    name=f"wpk{k}")
                    nc.sync.dma_start(out=wpk[:], in_=WPR[k])
                    wpks.append(wpk)

                # ---- Phase 2: attention, strip-level software pipeline ----
                # ScalarE's exp throughput (~16.7us/head) nearly equals the
                # PE's per-head matmul work (~17.6us), so neither engine can
                # afford to wait on the other.  AV chains run m-outer,
                # consuming each exp'd strip as it lands, interleaved with
                # the next strip-set's score pairs and the next head's q/k
                # projection chains; AV tiles c2 in {2,3} run as catch-up
                # blocks with no exp dependency left.
                with (
                    tc.tile_pool(name="estrip", bufs=3) as ep,
                    tc.tile_pool(name="fin", bufs=4) as fin,
                    tc.tile_pool(name="spool", bufs=2, space="PSUM") as spool,
                    tc.tile_pool(name="avps", bufs=3, space="PSUM") as avps,
                ):

                    def s_strip(qh, kh, e12, r, m):
                        """Score pair + exp for strip (r, m).  The two score
                        matmuls must hit different PSUM banks (concurrent
                        row-group writes to one bank fault); one exp covers
                        both."""
                        nsl = slice(r * 512, (r + 1) * 512)
                        msl = slice(m * 128, (m + 1) * 128)
                        sp = spool.tile([128, 2, 512], f32, tag="s")
                        nc.tensor.matmul(
                            sp[:, 0, :], kh[0:64, msl], qh[0:64, nsl],
                            start=True, stop=True,
                        )
                        nc.tensor.matmul(
                            sp[:, 1, :], kh[64:128, msl], qh[64:128, nsl],
                            start=True, stop=True,
                        )
                        nc.scalar.activation(
                            e12[:, m, :].rearrange("p (a b) -> p a b", a=2),
                            sp[:],
                            AF.Exp,
                            scale=SCALE,
                        )

                    def av_strip(h, o, e12, m, c2):
                        """One m-strip of both accumulation chains for AV
                        tile c2.  Both chains share one PSUM bank: only the
                        E1 chain's first matmul carries start=True (clears
                        has_written bank-wide); the E2 chain's first write
                        still overwrites because its bits are already
                        clear."""
                        nc.tensor.matmul(
                            o[:, 0:129],
                            e12[:, m, c2 * 128 : (c2 + 1) * 128],
                            v_aug[:, m, h, :],
                            start=(m == 0),
                            stop=(m == 7),
                            skip_group_check=True,
                        )
                        nc.tensor.matmul(
                            o[:, 129:258],
                            e12[:, m, 512 + c2 * 128 : 512 + (c2 + 1) * 128],
                            v_aug[:, m, h, :],
                            start=False,
                            stop=(m == 7),
                            skip_group_check=True,
                        )

                    def combine(h, r, c2, o):
                        """a1 - lam*a2 + LN stats for one finished AV tile
                        (VectorE, all per-partition)."""
                        jn = r * 4 + c2
                        r1 = fin.tile([128, 1], f32, tag="r1")
                        r2 = fin.tile([128, 1], f32, tag="r2")
                        nc.vector.reciprocal(r1[:], o[:, 128:129])
                        nc.vector.reciprocal(r2[:], o[:, 257:258])
                        t2 = fin.tile([128, 128], f32, tag="t2")
                        nc.vector.tensor_scalar(
                            t2[:], o[:, 129:257], r2[:], float(lam),
                            op0=OP.mult, op1=OP.mult,
                        )
                        nc.vector.scalar_tensor_tensor(
                            o_ln[:, jn, h, :],
                            o[:, 0:128],
                            r1[:],
                            t2[:],
                            op0=OP.mult,
                            op1=OP.subtract,
                        )
                        st6 = fin.tile([128, 6], f32, tag="st6")
                        nc.vector.bn_stats(st6[:], o_ln[:, jn, h, :])
                        nc.vector.bn_aggr(stats_sb[:, h, jn, :], st6[:])

                    # prologue: head 0's first strip-set
                    qh, kh = next_qk
                    e_r0 = ep.tile([128, 8, 1024], f16, tag="e")
                    for m in range(8):
                        s_strip(qh, kh, e_r0, 0, m)
                    if dbg:
                        nc.sync.dma_start(out=DE12[:], in_=e_r0[:])

                    for h in range(12):
                        if h + 1 < 12:
                            qh_n, kh_n, qk_chunks = qk_ops(h + 1)
                        else:
                            qh_n = kh_n = None
                            qk_chunks = []

                        # loop A: r1 scores + AV(r0) tiles {0,1} + qk chains
                        e_r1 = ep.tile([128, 8, 1024], f16, tag="e")
                        o_a = [
                            avps.tile([128, 258], f32, tag="o", name="o_av")
                            for _ in range(2)
                        ]
                        for m in range(8):
                            for c2 in range(2):
                                av_strip(h, o_a[c2], e_r0, m, c2)
                            # all 28 q/k ops land in loop A (4,4,4,4,3,3,3,3)
                            # so loop B's score strips never read q/k tiles
                            # ahead of the copies that produce them
                            for _ in range(4 if m < 4 else 3):
                                if qk_chunks:
                                    qk_chunks.pop(0)()
                            s_strip(qh, kh, e_r1, 1, m)
                        combine(h, 0, 0, o_a[0])
                        combine(h, 0, 1, o_a[1])
                        # catch-up: AV(r0) tiles {2,3} (exp all done)
                        o_b = [
                            avps.tile([128, 258], f32, tag="o", name="o_av")
                            for _ in range(2)
                        ]
                        for m in range(8):
                            for c2 in range(2):
                                av_strip(h, o_b[c2], e_r0, m, c2 + 2)
                        combine(h, 0, 2, o_b[0])
                        combine(h, 0, 3, o_b[1])

                        # loop B: next head's r0 scores + AV(r1) tiles {0,1}
                        e_n = None
                        if qh_n is not None:
                            e_n = ep.tile([128, 8, 1024], f16, tag="e")
                        o_c = [
                            avps.tile([128, 258], f32, tag="o", name="o_av")
                            for _ in range(2)
                        ]
                        for m in range(8):
                            for c2 in range(2):
                                av_strip(h, o_c[c2], e_r1, m, c2)
                            if e_n is not None:
                                s_strip(qh_n, kh_n, e_n, 0, m)
                        combine(h, 1, 0, o_c[0])
                        combine(h, 1, 1, o_c[1])
                        # catch-up: AV(r1) tiles {2,3}
                        o_d = [
                            avps.tile([128, 258], f32, tag="o", name="o_av")
                            for _ in range(2)
                        ]
                        for m in range(8):
                            for c2 in range(2):
                                av_strip(h, o_d[c2], e_r1, m, c2 + 2)
                        combine(h, 1, 2, o_d[0])
                        combine(h, 1, 3, o_d[1])

                        if dbg and h == 0:
                            nc.sync.dma_start(out=DQH[:], in_=qh[:])
                            nc.sync.dma_start(out=DKH[:], in_=kh[:])

                        # ---- per-head tail: rsqrt + LN apply ----
                        # rs = exp(-0.5 * ln(var + eps)); Exp and Ln share one
                        # pinned table set, so no reload happens here.
                        nc.scalar.activation(
                            sdbuf[:, h, :], stats_sb[:, h, :, 1],
                            AF.Ln, bias=eps_sb[:],
                        )
                        nc.scalar.activation(
                            rsbuf[:, h, :], sdbuf[:, h, :], AF.Exp, scale=-0.5
                        )
                        for jn in range(8):
                            nc.vector.tensor_scalar(
                                o_ln[:, jn, h, :],
                                o_ln[:, jn, h, :],
                                stats_sb[:, h, jn, 0:1],
                                rsbuf[:, h, jn : jn + 1],
                                op0=OP.subtract,
                                op1=OP.mult,
                            )

                        e_r0 = e_n
                        qh, kh = qh_n, kh_n
                    if dbg:
                        nc.sync.dma_start(out=DVAUG[:], in_=v_aug[:])
                        nc.sync.dma_start(out=DOLN[:], in_=o_ln[:])
                        nc.sync.dma_start(out=DSTATS[:], in_=stats_sb[:])

                _pools.close()

                # ---- tail: transposes interleaved with the projection.
                # Head k+1's transposes (LDW-bound) are emitted right after
                # head k's projection matmuls so their weight loads hide in
                # the PE reorder window; o_lnT[k+1] is ready (via the ScalarE
                # psum->sbuf copy) before the k+1 projection step needs it.
                if dbg:
                    nc.sync.dma_start(out=DOLNT[:], in_=o_lnT[:])
                with (
                    tc.tile_pool(name="tps", bufs=2, space="PSUM") as tps,
                    tc.tile_pool(name="tail", bufs=1) as tailp,
                    tc.tile_pool(name="fps", bufs=1, space="PSUM") as fps,
                ):
                    fout = tailp.tile([128, 6, 1024], f32)

                    def transpose_head(h):
                        for g2 in range(2):
                            tp = tps.tile([128, 4, 128], f16, tag="t")
                            for j in range(4):
                                jn = 4 * g2 + j
                                nc.tensor.transpose(
                                    tp[:, j, :], o_ln[:, jn, h, :], ident[:]
                                )
                            nc.scalar.copy(
                                o_lnT[:, h, g2 * 512 : (g2 + 1) * 512],
                                tp[:].rearrange("p a b -> p (a b)"),
                            )

                    transpose_head(0)
                    for g in range(2):
                        fs = {}
                        for mc in range(3 * g, 3 * g + 3):
                            for nr2 in range(2):
                                fs[(mc, nr2)] = fps.tile(
                                    [128, 512], f32, tag=f"f{mc % 3}_{nr2}",
                                    name=f"fpsum{mc}_{nr2}",
                                )
                        for k in range(12):
                            for mc in range(3 * g, 3 * g + 3):
                                for nr2 in range(2):
                                    nc.tensor.matmul(
                                        fs[(mc, nr2)][:],
                                        wpks[k][:, mc * 128 : (mc + 1) * 128],
                                        o_lnT[:, k, nr2 * 512 : (nr2 + 1) * 512],
                                        start=(k == 0),
                                        stop=(k == 11),
                                    )
                            if g == 0 and k + 1 < 12:
                                transpose_head(k + 1)
                        for mc in range(3 * g, 3 * g + 3):
                            for nr2 in range(2):
                                nsl2 = slice(nr2 * 512, (nr2 + 1) * 512)
                                nc.vector.tensor_scalar(
                                    fout[:, mc, nsl2],
                                    fs[(mc, nr2)][:],
                                    bpp_sb[:, mc : mc + 1],
                                    None,
                                    op0=OP.add,
                                )
                                nc.sync.dma_start(
                                    out=OUT[:, mc, nsl2],
                                    in_=fout[:, mc, nsl2],
                                )

    nc.compile()
    return nc


def _host_prep(x, Wq, Wk, Wv, gamma, beta, Wp, bp):
    x = np.ascontiguousarray(np.asarray(x, np.float32))
    Wq = np.asarray(Wq, np.float32)
    Wk = np.asarray(Wk, np.float32)
    Wv = np.asarray(Wv, np.float32)
    Wp = np.asarray(Wp, np.float32)
    bp = np.asarray(bp, np.float32)
    gamma = np.asarray(gamma, np.float32)
    beta = np.asarray(beta, np.float32)

    # xT per batch: [128, 6, 1024] with [p, k, n] = x[b, n, k*128+p]
    xTr = np.ascontiguousarray(
        x.transpose(0, 2, 1).reshape(B, 6, 128, N).transpose(0, 2, 1, 3)
    ).astype(np.float16)

    # W[qk]R: [12, 128, 6, 128] with [h, p, k, c] = W[k*128+p, h*128+c]
    def wqk_r(W):
        return np.ascontiguousarray(
            W.reshape(6, 128, 12, 128).transpose(2, 1, 0, 3)
        )

    WqR = wqk_r(Wq).astype(np.float16)
    WkR = wqk_r(Wk).astype(np.float16)
    # WvR: [128, 6, 1536] with [p, k, c] = Wv[k*128+p, c]
    WvR = np.ascontiguousarray(
        Wv.reshape(6, 128, 2 * C).transpose(1, 0, 2)
    ).astype(np.float16)
    # Fold gamma and the (1 - lambda_init) scale into Wp; beta into the bias.
    gfull = np.tile(gamma, H)  # [1536]
    Wpg = Wp * (OUT_SCALE * gfull)[:, None]
    bpp = bp + OUT_SCALE * (np.tile(beta, H) @ Wp)
    WpR = np.ascontiguousarray(Wpg.reshape(12, 128, C)).astype(np.float16)
    bppR = np.ascontiguousarray(bpp.reshape(6, 128).T)  # [128, 6]
    return xTr, WqR, WkR, WvR, WpR, bppR


def kernel(x, Wq, Wk, Wv, lam, gamma, beta, Wp, bp):
    global LAST_EXEC_NS, LAST_RESULTS
    import os

    from concourse.bass_utils import run_bass_kernel_spmd

    lam_f = float(np.asarray(lam))
    xTr, WqR, WkR, WvR, WpR, bppR = _host_prep(
        x, Wq, Wk, Wv, gamma, beta, Wp, bp
    )

    key = lam_f
    if key not in _BUILD_CACHE:
        _BUILD_CACHE[key] = _build(lam_f)
    nc = _BUILD_CACHE[key]

    in_maps = [
        {
            "xT": xTr[b],
            "WqR": WqR,
            "WkR": WkR,
            "WvR": WvR,
            "WpR": WpR,
            "bpp": bppR,
        }
        for b in range(B)
    ]

    trace = bool(os.environ.get("BASS_KERNEL_TRACE"))
    if trace:
        from concourse import bass_utils as _bu

        _bu.upload_artifacts = lambda tmpdir: "local://" + tmpdir
    res = run_bass_kernel_spmd(
        nc, in_maps, list(range(B)), trace=trace,
        **({"trace_cores": list(range(B))} if trace else {}),
    )
    LAST_EXEC_NS = res.exec_time_ns
    LAST_RESULTS = res

    out = np.empty((B, N, C), np.float32)
    for b in range(B):
        outT = res.results[b]["outT"]  # [128, 6, 1024]
        out[b] = outT.transpose(2, 1, 0).reshape(N, C)
    return out

